# revision 1
# baseline (speedup 1.0000x reference)
"""Multi-head attention (RoPE, non-causal) on 8 Trainium2 NeuronCores.

Problem: x[4,2048,2048] fp32; wq/wk/wv/wo [2048,2048]; biases [2048].
  q,k,v = x@w.T+b per 16 heads of dim 128; rope(q,k); softmax(q k^T/sqrt(128));
  out = (attn@v)@wo.T + bo.

Sharding: core c = 2*b + g -> batch b, head-group g (8 heads each).
Each core computes a partial output (its 8 heads) for its batch over the full
sequence; the host sums the pair partials (the wo contraction splits cleanly
over head groups) and adds bo_eff = bo + wo@bv (the V-bias folds out exactly
because softmax rows sum to 1).

Device program (SPMD, one NEFF, bf16 matmul operands / fp32 accumulation):
  V phase: V for all 8 local heads in natural [t, dh] layout
      (xT-chunk stationary, wv moving), spilled to DRAM as bf16.
  Fused per-head loop: Q^T/K^T projection (w-chunk stationary, xT moving)
      -> DVE bias+scale -> RoPE (DMA rotate-half + DVE mul/mul/add, bf16 out,
      stays in SBUF) -> flash-style attention over t-chunks in the transposed
      scores orientation [t, s]: scores matmul -> ACT exp (bf16) -> bf16 DVE
      partial sums for the softmax denominator + ctx^T accumulation in PSUM.
      Denominator finished with a ones-vector matmul (cross-partition sum),
      broadcast via a DRAM-bounce stride-0 DMA, applied with DVE.
      No max-subtraction: |scores| <= ~15 so exp is fp32-safe.
  P3: out[s, :] = sum_c ctxT_c^T @ woT_c over the core's 8 head-chunks.
"""

import sys

if "/opt/trn_rl_repo" not in sys.path:
    sys.path.insert(0, "/opt/trn_rl_repo")

import ml_dtypes
import numpy as np

import concourse.bass as bass
import concourse.tile as tile
from concourse import bacc, mybir
from concourse.bass_utils import run_bass_kernel_spmd

F32 = mybir.dt.float32
BF16 = mybir.dt.bfloat16
NPBF = ml_dtypes.bfloat16

B, S, D = 4, 2048, 2048
H = 16
DH = 128
HL = 8  # heads per core
KO = D // 128  # 16 k-chunks
TB = S // 128  # 16 t-chunks
ROPE_THETA = 10000.0
QSCALE = 1.0 / np.sqrt(DH)

_NC_CACHE = {}


def build_nc():
    nc = bacc.Bacc()

    xt_d = nc.declare_dram_parameter("xt", [KO, 128, S], BF16, isOutput=False)
    wq_d = nc.declare_dram_parameter("wq", [HL, KO, 128, 128], BF16, isOutput=False)
    wk_d = nc.declare_dram_parameter("wk", [HL, KO, 128, 128], BF16, isOutput=False)
    wv_d = nc.declare_dram_parameter("wv", [KO, 128, HL * DH], BF16, isOutput=False)
    wo_d = nc.declare_dram_parameter("wo", [HL, 128, D], BF16, isOutput=False)
    cos_d = nc.declare_dram_parameter("cosT", [128, S], F32, isOutput=False)
    sin_d = nc.declare_dram_parameter("sinS", [128, S], F32, isOutput=False)
    bq_d = nc.declare_dram_parameter("bq", [128, HL], F32, isOutput=False)
    bk_d = nc.declare_dram_parameter("bk", [128, HL], F32, isOutput=False)
    out_d = nc.declare_dram_parameter("out", [S, D], F32, isOutput=True)

    v_d = nc.dram_tensor("v_spill", [TB, 128, HL * DH], BF16)
    ct_d = nc.dram_tensor("ct_spill", [HL, 128, S], BF16)
    den_d = nc.dram_tensor("den_bounce", [HL, 1, S], F32)

    with tile.TileContext(nc) as tc:
        with (
            tc.tile_pool(name="xt_pool", bufs=1) as xt_pool,
            tc.tile_pool(name="const_pool", bufs=1) as const_pool,
            tc.tile_pool(name="psum_main", bufs=1, space="PSUM") as psum_main,
        ):
            xt_sb = xt_pool.tile([128, KO, S], BF16)
            nc.sync.dma_start(out=xt_sb[:], in_=xt_d[:].rearrange("k p s -> p k s"))

            cos_sb = const_pool.tile([128, S], F32)
            sin_sb = const_pool.tile([128, S], F32)
            bq_sb = const_pool.tile([128, HL], F32)
            bk_sb = const_pool.tile([128, HL], F32)
            ones_sb = const_pool.tile([128, 1], BF16)
            nc.sync.dma_start(out=cos_sb[:], in_=cos_d[:])
            nc.sync.dma_start(out=sin_sb[:], in_=sin_d[:])
            nc.sync.dma_start(out=bq_sb[:], in_=bq_d[:])
            nc.sync.dma_start(out=bk_sb[:], in_=bk_d[:])
            nc.vector.memset(ones_sb[:], 1.0)

            # ---------------- V phase: natural [t, dh] layout, all heads ----
            with (
                tc.tile_pool(name="wv_pool", bufs=1) as wv_pool,
                tc.tile_pool(name="vout_pool", bufs=3) as vout_pool,
            ):
                wv_sb = wv_pool.tile([128, KO, HL * DH], BF16)
                nc.sync.dma_start(
                    out=wv_sb[:], in_=wv_d[:].rearrange("k p m -> p k m")
                )
                for tb in range(TB):
                    for nf in range(2):
                        vps = psum_main.tile([128, 512], F32, tag="qk", bufs=2)
                        for ko in range(KO):
                            nc.tensor.matmul(
                                vps[:],
                                xt_sb[:, ko, tb * 128 : (tb + 1) * 128],
                                wv_sb[:, ko, nf * 512 : (nf + 1) * 512],
                                start=(ko == 0),
                                stop=(ko == KO - 1),
                            )
                        vsb = vout_pool.tile([128, 512], BF16)
                        nc.vector.tensor_copy(out=vsb[:], in_=vps[:])
                        nc.sync.dma_start(
                            out=v_d[tb, :, nf * 512 : (nf + 1) * 512], in_=vsb[:]
                        )

            # -------- fused per-head: QK projection + rope + attention ------
            with (
                tc.tile_pool(name="w_pool", bufs=2) as w_pool,
                tc.tile_pool(name="qs_pool", bufs=2) as qs_pool,
                tc.tile_pool(name="rot_pool", bufs=2) as rot_pool,
                tc.tile_pool(name="qb_pool", bufs=2) as qb_pool,
                tc.tile_pool(name="v2_pool", bufs=2) as v2_pool,
                tc.tile_pool(name="et_pool", bufs=4) as et_pool,
                tc.tile_pool(name="den_pool", bufs=2) as den_pool,
                tc.tile_pool(name="norm_pool", bufs=2) as norm_pool,
            ):
                for h in range(HL):
                    # --- Q^T and K^T for head h (rope'd, bf16, in SBUF)
                    qkb = []
                    for w_d, b_sb, scale in (
                        (wq_d, bq_sb, QSCALE),
                        (wk_d, bk_sb, 1.0),
                    ):
                        w_sb = w_pool.tile([128, KO, 128], BF16)
                        nc.sync.dma_start(
                            out=w_sb[:], in_=w_d[h].rearrange("k p m -> p k m")
                        )
                        qs = qs_pool.tile([128, S], F32)
                        for sh in range(2):  # s in two 1024 halves
                            qps = psum_main.tile([128, 1024], F32, tag="qk", bufs=2)
                            for ko in range(KO):
                                for n in range(2):
                                    nc.tensor.matmul(
                                        qps[:, n * 512 : (n + 1) * 512],
                                        w_sb[:, ko, :],
                                        xt_sb[
                                            :,
                                            ko,
                                            sh * 1024
                                            + n * 512 : sh * 1024
                                            + (n + 1) * 512,
                                        ],
                                        start=(ko == 0),
                                        stop=(ko == KO - 1),
                                    )
                            # qs = psum*scale + bias (per-partition)
                            nc.vector.tensor_scalar(
                                out=qs[:, sh * 1024 : (sh + 1) * 1024],
                                in0=qps[:],
                                scalar1=scale,
                                scalar2=b_sb[:, h : h + 1],
                                op0=mybir.AluOpType.mult,
                                op1=mybir.AluOpType.add,
                            )
                        # rope: q' = q*cos + rot(q)*sinS (sinS sign-folded)
                        rot = rot_pool.tile([128, S], F32)
                        nc.sync.dma_start(out=rot[0:64, :], in_=qs[64:128, :])
                        nc.sync.dma_start(out=rot[64:128, :], in_=qs[0:64, :])
                        nc.vector.tensor_mul(out=qs[:], in0=qs[:], in1=cos_sb[:])
                        nc.vector.tensor_mul(out=rot[:], in0=rot[:], in1=sin_sb[:])
                        qb = qb_pool.tile([128, S], BF16)
                        nc.vector.tensor_add(out=qb[:], in0=qs[:], in1=rot[:])
                        qkb.append(qb)
                    qt_sb, kt_sb = qkb

                    v_sb = v2_pool.tile([128, TB, DH], BF16)
                    nc.sync.dma_start(
                        out=v_sb[:],
                        in_=v_d[:, :, h * DH : (h + 1) * DH].rearrange(
                            "t p m -> p t m"
                        ),
                    )

                    # --- attention over t-chunks (scoresT orientation)
                    ctx_ps = psum_main.tile([128, S], F32, tag="ctx", bufs=1)
                    pden = den_pool.tile([128, S], BF16, tag="pden", bufs=2)
                    for tb in range(TB):
                        et = et_pool.tile([128, S], BF16)
                        for sh in range(2):
                            sc = psum_main.tile([128, 1024], F32, tag="qk", bufs=2)
                            for n in range(2):
                                nc.tensor.matmul(
                                    sc[:, n * 512 : (n + 1) * 512],
                                    kt_sb[:, tb * 128 : (tb + 1) * 128],
                                    qt_sb[
                                        :,
                                        sh * 1024 + n * 512 : sh * 1024 + (n + 1) * 512,
                                    ],
                                    start=True,
                                    stop=True,
                                )
                            nc.scalar.activation(
                                out=et[:, sh * 1024 : (sh + 1) * 1024],
                                in_=sc[:],
                                func=mybir.ActivationFunctionType.Exp,
                            )
                        if tb == 0:
                            nc.vector.tensor_copy(out=pden[:], in_=et[:])
                        else:
                            nc.vector.tensor_add(out=pden[:], in0=pden[:], in1=et[:])
                        for n in range(4):
                            nc.tensor.matmul(
                                ctx_ps[:, n * 512 : (n + 1) * 512],
                                v_sb[:, tb, :],
                                et[:, n * 512 : (n + 1) * 512],
                                start=(tb == 0),
                                stop=(tb == TB - 1),
                            )

                    # --- denominator: cross-partition sum via ones-matmul
                    den_sb = den_pool.tile([1, S], F32, tag="den", bufs=1)
                    for n in range(4):
                        dps = psum_main.tile([1, 512], F32, tag="qk", bufs=2)
                        nc.tensor.matmul(
                            dps[:],
                            ones_sb[:],
                            pden[:, n * 512 : (n + 1) * 512],
                            start=True,
                            stop=True,
                        )
                        nc.scalar.copy(
                            out=den_sb[:, n * 512 : (n + 1) * 512], in_=dps[:]
                        )
                    # fast PSUM release: copy unnormalized ctx^T to SBUF
                    cu = norm_pool.tile([128, S], F32, tag="cu", bufs=1)
                    nc.vector.tensor_copy(out=cu[:], in_=ctx_ps[:])
                    # broadcast den across partitions via DRAM bounce
                    nc.sync.dma_start(out=den_d[h], in_=den_sb[:])
                    bc = norm_pool.tile([128, S], F32, tag="bc")
                    den_ap = den_d[h]
                    bcast_src = bass.AP(
                        tensor=den_ap.tensor,
                        offset=den_ap.offset,
                        ap=[[0, 128]] + list(den_ap.ap[1:]),
                    )
                    nc.sync.dma_start(out=bc[:], in_=bcast_src)
                    nc.vector.reciprocal(out=bc[:], in_=bc[:])
                    ct_sb = norm_pool.tile([128, S], BF16, tag="ct")
                    nc.vector.tensor_mul(out=ct_sb[:], in0=cu[:], in1=bc[:])
                    nc.sync.dma_start(out=ct_d[h], in_=ct_sb[:])

        # ---------------- P3: output projection (partial) ----------------
        with (
            tc.tile_pool(name="wo_pool", bufs=1) as wo_pool,
            tc.tile_pool(name="ct_pool", bufs=2) as ct_pool,
            tc.tile_pool(name="out_pool", bufs=2) as out_pool,
            tc.tile_pool(name="psum_p3", bufs=8, space="PSUM") as psum_p3,
        ):
            wo_sb = wo_pool.tile([128, HL, D], BF16)
            nc.sync.dma_start(out=wo_sb[:], in_=wo_d[:].rearrange("c p m -> p c m"))
            for m in range(TB):
                cts = ct_pool.tile([128, HL, 128], BF16)
                nc.sync.dma_start(
                    out=cts[:],
                    in_=ct_d[:, :, m * 128 : (m + 1) * 128].rearrange(
                        "c p m2 -> p c m2"
                    ),
                )
                osb = out_pool.tile([128, D], F32)
                for n in range(4):
                    ops = psum_p3.tile([128, 512], F32)
                    for c in range(HL):
                        nc.tensor.matmul(
                            ops[:],
                            cts[:, c, :],
                            wo_sb[:, c, n * 512 : (n + 1) * 512],
                            start=(c == 0),
                            stop=(c == HL - 1),
                        )
                    nc.vector.tensor_copy(out=osb[:, n * 512 : (n + 1) * 512], in_=ops[:])
                nc.sync.dma_start(out=out_d[m * 128 : (m + 1) * 128, :], in_=osb[:])

    nc.finalize()
    return nc


def _get_nc():
    if "nc" not in _NC_CACHE:
        _NC_CACHE["nc"] = build_nc()
    return _NC_CACHE["nc"]


def _rope_tables():
    inv_freq = 1.0 / (ROPE_THETA ** (np.arange(0, DH, 2, dtype=np.float32) / DH))
    freqs = np.arange(S, dtype=np.float32)[:, None] * inv_freq[None, :]
    emb = np.concatenate([freqs, freqs], axis=-1)  # [S, 128]
    cosT = np.ascontiguousarray(np.cos(emb).T.astype(np.float32))  # [128, S]
    sinS = np.sin(emb).T.astype(np.float32).copy()
    sinS[0:64, :] *= -1.0  # sign-folded rotate_half
    return cosT, np.ascontiguousarray(sinS)


def kernel(x, wq, bq, wk, bk, wv, bv, wo, bo, _trace=False, _tmpdir=None):
    x = np.asarray(x, dtype=np.float32)
    wq = np.asarray(wq, dtype=np.float32)
    wk = np.asarray(wk, dtype=np.float32)
    wv = np.asarray(wv, dtype=np.float32)
    wo = np.asarray(wo, dtype=np.float32)
    bq = np.asarray(bq, dtype=np.float32)
    bk = np.asarray(bk, dtype=np.float32)
    bv = np.asarray(bv, dtype=np.float32)
    bo = np.asarray(bo, dtype=np.float32)

    nc = _get_nc()
    cosT, sinS = _rope_tables()

    def qk_pack(w, g):
        ws = w[g * 1024 : (g + 1) * 1024, :]
        return np.ascontiguousarray(
            ws.reshape(HL, 128, KO, 128).transpose(0, 2, 3, 1).astype(NPBF)
        )

    packs = []
    for g in range(2):
        wv_s = wv[g * 1024 : (g + 1) * 1024, :]
        wv_p = np.ascontiguousarray(
            wv_s.reshape(HL * DH, KO, 128).transpose(1, 2, 0).astype(NPBF)
        )
        wo_s = wo[:, g * 1024 : (g + 1) * 1024]
        wo_p = np.ascontiguousarray(
            wo_s.reshape(D, HL, 128).transpose(1, 2, 0).astype(NPBF)
        )
        bq_p = np.ascontiguousarray(
            (bq[g * 1024 : (g + 1) * 1024] * QSCALE).reshape(HL, 128).T
        )
        bk_p = np.ascontiguousarray(bk[g * 1024 : (g + 1) * 1024].reshape(HL, 128).T)
        packs.append(
            dict(
                wq=qk_pack(wq, g),
                wk=qk_pack(wk, g),
                wv=wv_p,
                wo=wo_p,
                bq=bq_p,
                bk=bk_p,
            )
        )

    in_maps = []
    xts = [
        np.ascontiguousarray(x[b].T.astype(NPBF)).reshape(KO, 128, S)
        for b in range(B)
    ]
    for c in range(8):
        b, g = c // 2, c % 2
        m = dict(packs[g])
        m["xt"] = xts[b]
        m["cosT"] = cosT
        m["sinS"] = sinS
        in_maps.append(m)

    res = run_bass_kernel_spmd(
        nc,
        in_maps,
        core_ids=list(range(8)),
        trace=_trace,
        tmpdir=_tmpdir,
    )

    bo_eff = bo + wo @ bv
    out = np.empty((B, S, D), dtype=np.float32)
    for b in range(B):
        out[b] = res.results[2 * b]["out"] + res.results[2 * b + 1]["out"]
        out[b] += bo_eff[None, :]
    if _trace:
        kernel.last_result = res
    return out



# revision 13
# speedup vs baseline: 1.3582x; 1.3582x over previous
"""Multi-head attention (RoPE, non-causal) on 8 Trainium2 NeuronCores — v2.

Problem: x[4,2048,2048] fp32; wq/wk/wv/wo [2048,2048]; biases [2048].
  q,k,v = x@w.T+b per 16 heads of dim 128; rope(q,k); softmax(q k^T/sqrt(128));
  out = (attn@v)@wo.T + bo.

Sharding: core c = 2*b + g -> batch b, head-group g (8 heads each).
Each core computes a partial output (its 8 heads) over the full sequence;
the host sums the pair partials and adds bo_eff = bo + wo@bv.

v2 changes vs v1 (1083us):
  - software-pipelined head loop: projection MMs for head h are interleaved
    at tb granularity with attention MMs of head h-1, with disjoint PSUM
    tags (proj/sc/ctx) so the in-order tensor queue never starves.
  - V and ct kept fully SBUF-resident (no DRAM spill/reload).
  - all DRAM operands host-packed partition-major: every DMA is
    128 descriptors of contiguous bytes (v1: ~84k descriptors).
  - softmax denominator reciprocal via reciprocal_approx_fast on the [1,S]
    row (v1: 12.9us DVE reciprocal on the broadcast [128,S] tile).
  - rope in bf16 at 2x DVE rate, in-place in the qb tile.
  - attention split in two s-halves of 1024 so ctx PSUM is double-buffered
    (2 banks each) and the den/normalize chain overlaps the next half.
"""

import sys

if "/opt/trn_rl_repo" not in sys.path:
    sys.path.insert(0, "/opt/trn_rl_repo")

from contextlib import ExitStack

import ml_dtypes
import numpy as np

import concourse.bass as bass
import concourse.tile as tile
from concourse import bacc, mybir
from concourse.bass_utils import run_bass_kernel_spmd

F32 = mybir.dt.float32
BF16 = mybir.dt.bfloat16
NPBF = ml_dtypes.bfloat16

B, S, D = 4, 2048, 2048
H = 16
DH = 128
HL = 8  # heads per core
KO = D // 128  # 16 k-chunks
TB = S // 128  # 16 t-chunks
ROPE_THETA = 10000.0
QSCALE = 1.0 / np.sqrt(DH)

_NC_CACHE = {}


def build_nc():
    nc = bacc.Bacc()

    xt_d = nc.declare_dram_parameter("xt", [128, KO, S], BF16, isOutput=False)
    wq_d = nc.declare_dram_parameter("wq", [HL, 128, KO, 128], BF16, isOutput=False)
    wk_d = nc.declare_dram_parameter("wk", [HL, 128, KO, 128], BF16, isOutput=False)
    wv_d = nc.declare_dram_parameter("wv", [128, KO, HL * DH], BF16, isOutput=False)
    wo_d = nc.declare_dram_parameter("wo", [128, HL, D], BF16, isOutput=False)
    cos_d = nc.declare_dram_parameter("cosT", [128, S], BF16, isOutput=False)
    sin_d = nc.declare_dram_parameter("sinS", [128, S], BF16, isOutput=False)
    bq_d = nc.declare_dram_parameter("bq", [128, HL], F32, isOutput=False)
    bk_d = nc.declare_dram_parameter("bk", [128, HL], F32, isOutput=False)
    out_d = nc.declare_dram_parameter("out", [S, D], F32, isOutput=True)

    den_d = nc.dram_tensor("den_bounce", [HL, 2, 1, 1024], F32)

    with tile.TileContext(nc) as tc, ExitStack() as es:
        const_pool = es.enter_context(tc.tile_pool(name="const", bufs=1))
        xt_pool = es.enter_context(tc.tile_pool(name="xt", bufs=1))
        vall_pool = es.enter_context(tc.tile_pool(name="vall", bufs=1))
        ctall_pool = es.enter_context(tc.tile_pool(name="ctall", bufs=1))
        psum = es.enter_context(tc.tile_pool(name="psum", bufs=1, space="PSUM"))

        xt_sb = xt_pool.tile([128, KO, S], BF16)
        for k in range(KO):
            nc.sync.dma_start(out=xt_sb[:, k, :], in_=xt_d[:, k, :])

        cos_sb = const_pool.tile([128, S], BF16)
        sin_sb = const_pool.tile([128, S], BF16)
        bq_sb = const_pool.tile([128, HL], F32)
        bk_sb = const_pool.tile([128, HL], F32)
        ones_sb = const_pool.tile([128, 1], BF16)
        nc.sync.dma_start(out=cos_sb[:], in_=cos_d[:])
        nc.sync.dma_start(out=sin_sb[:], in_=sin_d[:])
        nc.sync.dma_start(out=bq_sb[:], in_=bq_d[:])
        nc.sync.dma_start(out=bk_sb[:], in_=bk_d[:])
        nc.vector.memset(ones_sb[:], 1.0)

        vall = vall_pool.tile([128, TB, HL * DH], BF16)
        ct_all = ctall_pool.tile([128, HL, S], BF16)

        # ---------------- V phase: v^T per tb in [t, m] layout --------------
        with tc.tile_pool(name="wv", bufs=1) as wv_pool:
            wv_sb = wv_pool.tile([128, KO, HL * DH], BF16)
            nc.sync.dma_start(out=wv_sb[:], in_=wv_d[:])
            for tb in range(TB):
                vp = psum.tile([128, 1024], F32, tag="proj", bufs=2)
                for ko in range(KO):
                    st, sp = (ko == 0), (ko == KO - 1)
                    nc.tensor.matmul(
                        vp[:, 0:512],
                        xt_sb[:, ko, tb * 128 : (tb + 1) * 128],
                        wv_sb[:, ko, 0:512],
                        start=st,
                        stop=sp,
                    )
                    nc.tensor.matmul(
                        vp[:, 512:1024],
                        xt_sb[:, ko, tb * 128 : (tb + 1) * 128],
                        wv_sb[:, ko, 512:1024],
                        start=st,
                        stop=sp,
                    )
                nc.scalar.copy(out=vall[:, tb, :], in_=vp[:])

        # -------- fused pipelined per-head loop ------------------------------
        with (
            tc.tile_pool(name="w", bufs=3) as w_pool,
            tc.tile_pool(name="rot", bufs=2) as rot_pool,
            tc.tile_pool(name="qb", bufs=4) as qb_pool,
            tc.tile_pool(name="et", bufs=3) as et_pool,
            tc.tile_pool(name="pden", bufs=2) as pden_pool,
            tc.tile_pool(name="bc", bufs=1) as bc_pool,
            tc.tile_pool(name="cu", bufs=2) as cu_pool,
            tc.tile_pool(name="den", bufs=2) as den_pool,
            tc.tile_pool(name="rden", bufs=2) as rden_pool,
        ):
            # per-head persistent state
            wsb = {}  # (h, wi) -> w tile
            qkb = {}  # (h, wi) -> rope'd bf16 [128, S] (wi: 0=k, 1=q)
            pstate = {}  # current proj unit: (h, wi) -> (psA, psB)

            def issue_w_loads(h):
                for wi, wd in ((0, wk_d), (1, wq_d)):
                    t = w_pool.tile([128, KO, 128], BF16, name=f"w{h}_{wi}", tag="w")
                    nc.sync.dma_start(out=t[:], in_=wd[h])
                    wsb[(h, wi)] = t

            def issue_proj_ko(h, s):
                # slot s: 0..15 -> K unit ko=s; 16..31 -> Q unit ko=s-16.
                # 4 MMs per ko share one stationary load; psA/psB cover S.
                wi, ko = (0, s) if s < TB else (1, s - TB)
                w_sb = wsb[(h, wi)]
                if ko == 0:
                    qkb[(h, wi)] = qb_pool.tile(
                        [128, S], BF16, name=f"qb{h}_{wi}", tag="qb"
                    )
                    psA = psum.tile([128, 1024], F32, tag="proj", bufs=2, name="psA")
                    psB = psum.tile([128, 1024], F32, tag="proj", bufs=2, name="psB")
                    pstate[(h, wi)] = (psA, psB)
                psA, psB = pstate[(h, wi)]
                st, sp = (ko == 0), (ko == KO - 1)
                for j, ps in enumerate((psA, psA, psB, psB)):
                    nc.tensor.matmul(
                        ps[:, (j % 2) * 512 : (j % 2 + 1) * 512],
                        w_sb[:, ko, :],
                        xt_sb[:, ko, j * 512 : (j + 1) * 512],
                        start=st,
                        stop=sp,
                    )
                if not sp:
                    return
                # unit complete: cast+bias per half, then rope in place
                scale = 1.0 if wi == 0 else QSCALE
                b_sb = bk_sb if wi == 0 else bq_sb
                qb = qkb[(h, wi)]
                del pstate[(h, wi)]
                for half, ps in enumerate((psA, psB)):
                    lo, hi = half * 1024, (half + 1) * 1024
                    nc.vector.tensor_scalar(
                        out=qb[:, lo:hi],
                        in0=ps[:],
                        scalar1=scale,
                        scalar2=b_sb[:, h : h + 1],
                        op0=mybir.AluOpType.mult,
                        op1=mybir.AluOpType.add,
                    )
                    rot = rot_pool.tile([128, 1024], BF16)
                    nc.sync.dma_start(out=rot[0:64, :], in_=qb[64:128, lo:hi])
                    nc.sync.dma_start(out=rot[64:128, :], in_=qb[0:64, lo:hi])
                    nc.vector.tensor_mul(
                        out=rot[:], in0=rot[:], in1=sin_sb[:, lo:hi]
                    )
                    nc.vector.tensor_mul(
                        out=qb[:, lo:hi], in0=qb[:, lo:hi], in1=cos_sb[:, lo:hi]
                    )
                    nc.vector.tensor_add(
                        out=qb[:, lo:hi], in0=qb[:, lo:hi], in1=rot[:]
                    )

            # attention state for the in-flight head
            att = {}

            def att_start(hh):
                att.clear()
                att["h"] = hh
                att["ctx"] = {}
                att["pden"] = {}
                att["prev"] = None

            def att_ctx_mm(hh, half, tb, et):
                cp = att["ctx"][half]
                st, sp = (tb == 0), (tb == TB - 1)
                vs = vall[:, tb, hh * DH : (hh + 1) * DH]
                nc.tensor.matmul(cp[:, 0:512], vs, et[:, 0:512], start=st, stop=sp)
                nc.tensor.matmul(cp[:, 512:1024], vs, et[:, 512:1024], start=st, stop=sp)

            def att_den_norm(hh, half):
                # release ctx PSUM promptly (bufs=1): copy to SBUF first
                cu = cu_pool.tile([128, 1024], F32)
                nc.vector.tensor_copy(out=cu[:], in_=att["ctx"][half][:])
                # denominator: ones-matmul partition sum of pden -> [1, 1024]
                pden = att["pden"][half]
                den_row = den_pool.tile([1, 1024], F32)
                for j in range(2):
                    dp = psum.tile([1, 512], F32, tag="sc", bufs=2)
                    nc.tensor.matmul(
                        dp[:],
                        ones_sb[:],
                        pden[:, j * 512 : (j + 1) * 512],
                        start=True,
                        stop=True,
                    )
                    nc.scalar.copy(out=den_row[:, j * 512 : (j + 1) * 512], in_=dp[:])
                rden = rden_pool.tile([1, 1024], F32)
                nc.vector.reciprocal_approx_fast(rden[:], den_row[:])
                nc.sync.dma_start(out=den_d[hh, half], in_=rden[:])
                bc = bc_pool.tile([128, 1024], F32)
                den_ap = den_d[hh, half]
                bcast_src = bass.AP(
                    tensor=den_ap.tensor,
                    offset=den_ap.offset,
                    ap=[[0, 128]] + list(den_ap.ap[1:]),
                )
                nc.sync.dma_start(out=bc[:], in_=bcast_src)
                nc.vector.tensor_mul(
                    out=ct_all[:, hh, half * 1024 : (half + 1) * 1024],
                    in0=cu[:],
                    in1=bc[:],
                )

            def att_slot(s):
                hh = att["h"]
                half, tb = divmod(s, TB)
                kb = qkb[(hh, 0)]
                qb = qkb[(hh, 1)]
                if tb == 0:
                    att["ctx"][half] = psum.tile(
                        [128, 1024], F32, tag="ctx", bufs=1, name=f"ctxps{half}"
                    )
                # scores for (half, tb)
                et = et_pool.tile([128, 1024], BF16)
                kt = kb[:, tb * 128 : (tb + 1) * 128]
                for j in range(2):
                    sc = psum.tile([128, 512], F32, tag="sc", bufs=2)
                    nc.tensor.matmul(
                        sc[:],
                        kt,
                        qb[:, half * 1024 + j * 512 : half * 1024 + (j + 1) * 512],
                        start=True,
                        stop=True,
                    )
                    nc.scalar.activation(
                        out=et[:, j * 512 : (j + 1) * 512],
                        in_=sc[:],
                        func=mybir.ActivationFunctionType.Exp,
                    )
                if tb == 0:
                    pden = pden_pool.tile([128, 1024], BF16)
                    att["pden"][half] = pden
                    nc.vector.tensor_copy(out=pden[:], in_=et[:])
                else:
                    pden = att["pden"][half]
                    nc.vector.tensor_add(out=pden[:], in0=pden[:], in1=et[:])
                # lagged ctx accumulation for the previous slot
                if att["prev"] is not None:
                    ph, ptb, pet = att["prev"]
                    att_ctx_mm(hh, ph, ptb, pet)
                    if ptb == TB - 1:
                        att_den_norm(hh, ph)
                att["prev"] = (half, tb, et)

            def att_flush():
                hh = att["h"]
                ph, ptb, pet = att["prev"]
                att_ctx_mm(hh, ph, ptb, pet)
                att_den_norm(hh, ph)
                att["prev"] = None
                # release references so qb pool bufs recycle
                del qkb[(hh, 0)], qkb[(hh, 1)]

            issue_w_loads(0)
            for h in range(HL + 1):
                for s in range(2 * TB):
                    if h < HL:
                        issue_proj_ko(h, s)
                    if h < HL and s == 16 and h + 1 < HL:
                        issue_w_loads(h + 1)
                    if h >= 1:
                        att_slot(s)
                if h >= 1:
                    att_flush()
                if h < HL:
                    att_start(h)

        # ---------------- P3: output projection (partial) -------------------
        with (
            tc.tile_pool(name="wo", bufs=1) as wo_pool,
            tc.tile_pool(name="osb", bufs=2) as osb_pool,
        ):
            wo_sb = wo_pool.tile([128, HL, D], BF16)
            for c in range(HL):
                nc.sync.dma_start(out=wo_sb[:, c, :], in_=wo_d[:, c, :])
            for m in range(TB):
                osb = osb_pool.tile([128, D], F32)
                p0 = psum.tile([128, 1024], F32, tag="proj", bufs=2)
                p1 = psum.tile([128, 1024], F32, tag="proj", bufs=2)
                for c in range(HL):
                    st, sp = (c == 0), (c == HL - 1)
                    cts = ct_all[:, c, m * 128 : (m + 1) * 128]
                    nc.tensor.matmul(p0[:, 0:512], cts, wo_sb[:, c, 0:512], start=st, stop=sp)
                    nc.tensor.matmul(p0[:, 512:1024], cts, wo_sb[:, c, 512:1024], start=st, stop=sp)
                    nc.tensor.matmul(p1[:, 0:512], cts, wo_sb[:, c, 1024:1536], start=st, stop=sp)
                    nc.tensor.matmul(p1[:, 512:1024], cts, wo_sb[:, c, 1536:2048], start=st, stop=sp)
                nc.vector.tensor_copy(out=osb[:, 0:1024], in_=p0[:])
                nc.vector.tensor_copy(out=osb[:, 1024:2048], in_=p1[:])
                nc.sync.dma_start(out=out_d[m * 128 : (m + 1) * 128, :], in_=osb[:])

    nc.finalize()
    return nc


def _get_nc():
    if "nc" not in _NC_CACHE:
        _NC_CACHE["nc"] = build_nc()
    return _NC_CACHE["nc"]


def _rope_tables():
    inv_freq = 1.0 / (ROPE_THETA ** (np.arange(0, DH, 2, dtype=np.float32) / DH))
    freqs = np.arange(S, dtype=np.float32)[:, None] * inv_freq[None, :]
    emb = np.concatenate([freqs, freqs], axis=-1)  # [S, 128]
    cosT = np.ascontiguousarray(np.cos(emb).T.astype(NPBF))  # [128, S]
    sinS = np.sin(emb).T.astype(np.float32).copy()
    sinS[0:64, :] *= -1.0  # sign-folded rotate_half
    return cosT, np.ascontiguousarray(sinS.astype(NPBF))


def kernel(x, wq, bq, wk, bk, wv, bv, wo, bo, _trace=False, _tmpdir=None):
    x = np.asarray(x, dtype=np.float32)
    wq = np.asarray(wq, dtype=np.float32)
    wk = np.asarray(wk, dtype=np.float32)
    wv = np.asarray(wv, dtype=np.float32)
    wo = np.asarray(wo, dtype=np.float32)
    bq = np.asarray(bq, dtype=np.float32)
    bk = np.asarray(bk, dtype=np.float32)
    bv = np.asarray(bv, dtype=np.float32)
    bo = np.asarray(bo, dtype=np.float32)

    nc = _get_nc()
    cosT, sinS = _rope_tables()

    def qk_pack(w, g):
        # [HL, 128(p), KO, 128(m)]: w_sb[p,k,m] = w[g*1024+h*128+m, k*128+p]
        hs = []
        for h in range(HL):
            W = w[g * 1024 + h * 128 : g * 1024 + (h + 1) * 128, :]  # [m, din]
            hs.append(W.T.reshape(KO, 128, 128).transpose(1, 0, 2))
        return np.ascontiguousarray(np.stack(hs, axis=0).astype(NPBF))

    packs = []
    for g in range(2):
        V = wv[g * 1024 : (g + 1) * 1024, :]  # [m', din]
        wv_p = np.ascontiguousarray(
            V.T.reshape(KO, 128, HL * DH).transpose(1, 0, 2).astype(NPBF)
        )
        WO = wo[:, g * 1024 : (g + 1) * 1024]  # [n, dh-block]
        wo_p = np.ascontiguousarray(
            WO.T.reshape(HL, 128, D).transpose(1, 0, 2).astype(NPBF)
        )
        bq_p = np.ascontiguousarray(
            (bq[g * 1024 : (g + 1) * 1024] * QSCALE).reshape(HL, 128).T
        )
        bk_p = np.ascontiguousarray(bk[g * 1024 : (g + 1) * 1024].reshape(HL, 128).T)
        packs.append(
            dict(wq=qk_pack(wq, g), wk=qk_pack(wk, g), wv=wv_p, wo=wo_p, bq=bq_p, bk=bk_p)
        )

    xts = [
        np.ascontiguousarray(
            x[b].T.reshape(KO, 128, S).transpose(1, 0, 2).astype(NPBF)
        )
        for b in range(B)
    ]
    in_maps = []
    for c in range(8):
        b, g = c // 2, c % 2
        m = dict(packs[g])
        m["xt"] = xts[b]
        m["cosT"] = cosT
        m["sinS"] = sinS
        in_maps.append(m)

    res = run_bass_kernel_spmd(
        nc,
        in_maps,
        core_ids=list(range(8)),
        trace=_trace,
        tmpdir=_tmpdir,
    )

    bo_eff = bo + wo @ bv
    out = np.empty((B, S, D), dtype=np.float32)
    for b in range(B):
        out[b] = res.results[2 * b]["out"] + res.results[2 * b + 1]["out"]
        out[b] += bo_eff[None, :]
    if _trace:
        kernel.last_result = res
    return out
